# revision 38
# baseline (speedup 1.0000x reference)
"""Trainium2 Bass kernel for 3-layer EGAT message passing (nn_COUNTYOD).

Strategy (8 cores, edge parallelism by dst range):
  - Host: sort edges by dst; device d owns dst nodes [d*6272, (d+1)*6272);
    edges grouped into 49 blocks of 128 dst nodes. Within a block, edges are
    split into two groups by src chunk (first 24 / last 25 blocks of the
    src's owner device), each padded to a tile multiple (TA/TB tiles).
  - Gather tables catA/catB hold rows [nh@Wni + b | nh@Wns] (192 f32 =
    768B, %256==0) in chunk-major order, so a whole block's src rows are
    fetched with TWO dma_gather instructions (int16 idxs, -1 suffix pad,
    per-device counts via reg_load) instead of per-tile indirect DMAs.
  - nj[dst] is block-local: njb = nh_sliceT[block] @ Wnj once per block,
    then selected per tile with a transposed one-hot matmul on PE.
  - Single fused pass per block: f_pre (PSUM) -> leaky (Act Prelu) ->
    logits (mul+grouped reduce) -> ex=exp(min(e,60)) -> msg=ns*ex (bf16)
    -> one-hot scatter matmul accumulating [h | s] in PSUM -> h = ps/s.
  - Edge features stored row-major bf16 [E,96]; next layer loads them
    transposed via dma_start_transpose. All per-tile matmuls in bf16.
  - Layer-0 tables are host-precomputed params (no collective); layers 1-2
    AllGather the locally computed 192-col table in two chunks.
"""

import sys
import numpy as np

for _p in ("/opt/trn_rl_repo",):
    if _p not in sys.path:
        sys.path.insert(0, _p)

import ml_dtypes  # noqa: E402
import concourse.bass as bass  # noqa: E402
import concourse.bacc as bacc  # noqa: E402
import concourse.mybir as mybir  # noqa: E402
import concourse.tile as tile  # noqa: E402
from concourse.masks import make_identity  # noqa: E402

F32 = mybir.dt.float32
BF16 = mybir.dt.bfloat16
I32 = mybir.dt.int32
I16 = mybir.dt.int16
AF = mybir.ActivationFunctionType
ALU = mybir.AluOpType
BNP = ml_dtypes.bfloat16

P = 128
FD = 96           # H*HE
CW = 192          # gather row width [ni|ns]
H = 3
HE = 32
EPS = 1e-20
N_REAL = 50000
NDEV = 8
NBLK = 49
R = NBLK * P      # 6272 nodes per device
NPAD = NDEV * R
CBLK = (24, 25)   # blocks per src chunk (within device)
CROWS = (CBLK[0] * P, CBLK[1] * P)          # 3072, 3200
CTOT = (NDEV * CROWS[0], NDEV * CROWS[1])   # 24576, 25600 rows per table


GSUB = 8          # max tiles (1024 idxs) per dma_gather (SWDGE ring limit)


def _subs(Tn):
    """Split Tn tiles into sub-gather spans of <= GSUB tiles."""
    out = []
    t0 = 0
    while t0 < Tn:
        out.append((t0, min(GSUB, Tn - t0)))
        t0 += GSUB
    return out


class Cfg:
    def __init__(self, TA, TB, odf, tc=4):
        self.TA = TA
        self.TB = TB
        self.T = TA + TB
        self.odf = odf
        self.tc = tc
        self.ttot = NBLK * self.T
        self.epd = self.ttot * P
        self.subsA = _subs(TA)
        self.subsB = _subs(TB)
        self.nsub = len(self.subsA) + len(self.subsB)


def _wrap16(v):
    """idx list -> [16, n/16] layout (value i at [i%16, i//16])."""
    return np.ascontiguousarray(v.reshape(-1, 16).T)


def host_prep(inputs, cfg=None):
    src = np.asarray(inputs["src"]).astype(np.int64)
    dst = np.asarray(inputs["dst"]).astype(np.int64)
    E = src.shape[0]

    order = np.argsort(dst, kind="stable")
    ssrc, sdst = src[order], dst[order]
    ef0 = np.asarray(inputs["countyodfeats"]).astype(np.float32)[order]
    odf = ef0.shape[1]

    # chunk id + table row for each (sorted) edge's src
    d_s = ssrc // R
    r_s = ssrc % R
    k_s = (r_s >= CROWS[0]).astype(np.int64)
    row_s = np.where(k_s == 0, d_s * CROWS[0] + r_s,
                     d_s * CROWS[1] + (r_s - CROWS[0]))

    blk = sdst // P
    blkcnt = np.bincount(blk, minlength=NPAD // P)
    starts = np.zeros(NPAD // P + 1, np.int64)
    np.cumsum(blkcnt, out=starts[1:])

    # per (dev, blk) group sizes to fix TA/TB
    nlo = np.zeros((NDEV, NBLK), np.int64)
    nhi = np.zeros((NDEV, NBLK), np.int64)
    for d in range(NDEV):
        for b in range(NBLK):
            g = d * NBLK + b
            ks = k_s[starts[g]:starts[g + 1]]
            nlo[d, b] = int((ks == 0).sum())
            nhi[d, b] = int((ks == 1).sum())
    TA = max(1, int(np.ceil(nlo.max() / P)))
    TB = max(1, int(np.ceil(nhi.max() / P)))
    if cfg is None:
        cfg = Cfg(TA, TB, odf)
    else:
        assert cfg.TA >= TA and cfg.TB >= TB
    T, TP = cfg.T, cfg.T * P

    def subcnt(n, t0, tn):
        """valid count within sub-gather span [t0*P, t0*P+tn*P) of a
        region with n valid prefix entries; >=1 (a forced idx 0 covers 0)."""
        return max(1, min(n - t0 * P, tn * P)) if n > t0 * P else 1

    percore = []
    for d in range(NDEV):
        idxA = np.full((NBLK, cfg.TA * P), -1, np.int64)
        idxB = np.full((NBLK, cfg.TB * P), -1, np.int64)
        cnts = np.zeros((1, cfg.nsub * NBLK), np.int32)
        dloc = np.full((NBLK, TP), -1.0, np.float32)
        efp = np.zeros((NBLK, TP, odf), np.float32)
        for b in range(NBLK):
            g = d * NBLK + b
            s0, s1 = starts[g], starts[g + 1]
            ks = k_s[s0:s1]
            lo = np.nonzero(ks == 0)[0]
            hi = np.nonzero(ks == 1)[0]
            na, nb = len(lo), len(hi)
            idxA[b, :na] = row_s[s0:s1][lo]
            idxB[b, :nb] = row_s[s0:s1][hi]
            ci = b * cfg.nsub
            for (t0, tn) in cfg.subsA:
                cnts[0, ci] = subcnt(na, t0, tn)
                if na <= t0 * P:
                    idxA[b, t0 * P] = 0
                ci += 1
            for (t0, tn) in cfg.subsB:
                cnts[0, ci] = subcnt(nb, t0, tn)
                if nb <= t0 * P:
                    idxB[b, t0 * P] = 0
                ci += 1
            dloc[b, :na] = (sdst[s0:s1][lo] - g * P).astype(np.float32)
            dloc[b, cfg.TA * P : cfg.TA * P + nb] = (
                sdst[s0:s1][hi] - g * P
            ).astype(np.float32)
            efp[b, :na] = ef0[s0:s1][lo]
            efp[b, cfg.TA * P : cfg.TA * P + nb] = ef0[s0:s1][hi]
        ia = np.concatenate([_wrap16(idxA[b]) for b in range(NBLK)], axis=1)
        ib = np.concatenate([_wrap16(idxB[b]) for b in range(NBLK)], axis=1)
        dstloc = dloc.reshape(-1, P).T            # [128, ttot]
        dstlocT = dloc.reshape(1, -1)             # [1, epd]
        percore.append(
            dict(
                idxA=np.ascontiguousarray(np.tile(ia, (8, 1)).astype(np.int16)),
                idxB=np.ascontiguousarray(np.tile(ib, (8, 1)).astype(np.int16)),
                cnts=cnts,
                dstloc=np.ascontiguousarray(dstloc),
                dstlocT=np.ascontiguousarray(dstlocT.astype(BNP)),
                ef0T=np.ascontiguousarray(
                    efp.reshape(-1, odf).T.astype(BNP)
                ),
            )
        )

    g = lambda k: np.asarray(inputs[k]).astype(np.float32)
    nh0 = g("ndata_h")
    nh0p = np.zeros((NPAD, FD), np.float32)
    nh0p[:N_REAL] = nh0[:N_REAL]
    for d in range(NDEV):
        sl = nh0p[d * R : (d + 1) * R]
        percore[d]["nh0T"] = np.ascontiguousarray(
            np.concatenate([sl.T, np.ones((1, R), np.float32)], axis=0)
        )

    # host-precomputed layer-0 gather tables (chunk-major row order)
    ni0 = nh0p @ g("Wni0") + g("bias0")[None, :]
    ns0 = nh0p @ g("Wns0")
    cat0 = np.concatenate([ni0, ns0], axis=1).astype(np.float32)  # [NPAD,192]
    c4 = cat0.reshape(NDEV, R, CW)
    catA0 = np.ascontiguousarray(c4[:, : CROWS[0]].reshape(CTOT[0], CW))
    catB0 = np.ascontiguousarray(c4[:, CROWS[0] :].reshape(CTOT[1], CW))

    def wcat(Wni, Wns, bias):
        w = np.concatenate([Wni, Wns], axis=1)
        b = np.concatenate([bias, np.zeros(FD, np.float32)])[None, :]
        return np.ascontiguousarray(
            np.concatenate([w, b], axis=0).astype(np.float32)
        )

    def wnj(W):
        return np.ascontiguousarray(
            np.concatenate([W, np.zeros((1, FD), np.float32)], axis=0)
        )

    weights = dict(
        catA0=catA0,
        catB0=catB0,
        wcat1=wcat(g("Wni")[0], g("Wns")[0], g("bias")[0]),
        wcat2=wcat(g("Wni")[1], g("Wns")[1], g("bias")[1]),
        wnj0=wnj(g("Wnj0")),
        wnj1=wnj(g("Wnj")[0]),
        wnj2=wnj(g("Wnj")[1]),
        wfij0=np.ascontiguousarray(g("Wfij0").astype(BNP)),
        wfij1=np.ascontiguousarray(g("Wfij")[0].astype(BNP)),
        wfij2=np.ascontiguousarray(g("Wfij")[1].astype(BNP)),
        attn0=np.ascontiguousarray(np.repeat(g("attn0").reshape(1, FD), P, 0)),
        attn1=np.ascontiguousarray(
            np.repeat(g("attn").reshape(2, FD)[0:1], P, 0)
        ),
        attn2=np.ascontiguousarray(
            np.repeat(g("attn").reshape(2, FD)[1:2], P, 0)
        ),
    )
    for d in range(NDEV):
        percore[d].update(weights)
    return percore, cfg


def build_program(cfg, nlayers=3, dbg=False, stage=99):
    nc = bacc.Bacc("TRN2", target_bir_lowering=False, debug=False)
    c = cfg
    T, TA, TB, TTOT, EPD = c.T, c.TA, c.TB, c.ttot, c.epd

    # ---- I/O ----
    pr = {}
    pr["nh0T"] = nc.declare_dram_parameter("nh0T", [FD + 1, R], F32, isOutput=False)
    pr["ef0T"] = nc.declare_dram_parameter("ef0T", [c.odf, EPD], BF16, isOutput=False)
    pr["idxA"] = nc.declare_dram_parameter("idxA", [P, NBLK * TA * 8], I16, isOutput=False)
    pr["idxB"] = nc.declare_dram_parameter("idxB", [P, NBLK * TB * 8], I16, isOutput=False)
    pr["cnts"] = nc.declare_dram_parameter("cnts", [1, cfg.nsub * NBLK], I32, isOutput=False)
    pr["dstloc"] = nc.declare_dram_parameter("dstloc", [P, TTOT], F32, isOutput=False)
    pr["dstlocT"] = nc.declare_dram_parameter("dstlocT", [1, EPD], BF16, isOutput=False)
    pr["catA0"] = nc.declare_dram_parameter("catA0", [CTOT[0], CW], F32, isOutput=False)
    pr["catB0"] = nc.declare_dram_parameter("catB0", [CTOT[1], CW], F32, isOutput=False)
    for i in (1, 2):
        pr[f"wcat{i}"] = nc.declare_dram_parameter(f"wcat{i}", [FD + 1, CW], F32, isOutput=False)
    for i in range(3):
        pr[f"wnj{i}"] = nc.declare_dram_parameter(f"wnj{i}", [FD + 1, FD], F32, isOutput=False)
        pr[f"attn{i}"] = nc.declare_dram_parameter(f"attn{i}", [P, FD], F32, isOutput=False)
    pr["wfij0"] = nc.declare_dram_parameter("wfij0", [c.odf, FD], BF16, isOutput=False)
    pr["wfij1"] = nc.declare_dram_parameter("wfij1", [FD, FD], BF16, isOutput=False)
    pr["wfij2"] = nc.declare_dram_parameter("wfij2", [FD, FD], BF16, isOutput=False)
    out3 = nc.declare_dram_parameter("out3", [R, FD], F32, isOutput=True)
    dbg_t = {}
    if dbg:
        dbg_t["d_nh"] = nc.declare_dram_parameter("d_nh", [FD + 1, R], F32, isOutput=True)
        dbg_t["d_G"] = nc.declare_dram_parameter("d_G", [P, cfg.T * CW], F32, isOutput=True)
        dbg_t["d_eL"] = nc.declare_dram_parameter("d_eL", [P, cfg.T * H], F32, isOutput=True)
        dbg_t["d_t2"] = nc.declare_dram_parameter("d_t2", [P, cfg.T * FD], F32, isOutput=True)

    # ---- internal DRAM ----
    efA = nc.dram_tensor("efA", [EPD, FD], BF16)
    efB = nc.dram_tensor("efB", [EPD, FD], BF16)
    catL = nc.dram_tensor("catL", [R, CW], F32)
    catG = {
        1: (
            nc.dram_tensor("catGA1", [CTOT[0], CW], F32, addr_space="Shared"),
            nc.dram_tensor("catGB1", [CTOT[1], CW], F32, addr_space="Shared"),
        ),
        2: (
            nc.dram_tensor("catGA2", [CTOT[0], CW], F32, addr_space="Shared"),
            nc.dram_tensor("catGB2", [CTOT[1], CW], F32, addr_space="Shared"),
        ),
    }

    rg = [list(range(NDEV))]

    with tile.TileContext(nc) as tc:
        with tc.tile_pool(name="persist", bufs=1) as pp:
            ident = pp.tile([P, P], F32, tag="ident")
            make_identity(nc, ident[:])
            iota_i = pp.tile([P, P], I32, tag="iota_i")
            nc.gpsimd.iota(iota_i[:], pattern=[[1, P]], base=0, channel_multiplier=0)
            iota_f = pp.tile([P, P], F32, tag="iota_f")
            nc.vector.tensor_copy(out=iota_f[:], in_=iota_i[:])
            iota_ci = pp.tile([P, 1], I32, tag="iota_ci")
            nc.gpsimd.iota(iota_ci[:], pattern=[[0, 1]], base=0, channel_multiplier=1)
            iota_c = pp.tile([P, 1], F32, tag="iota_c")
            nc.vector.tensor_copy(out=iota_c[:], in_=iota_ci[:])
            ones1 = pp.tile([1, P], BF16, tag="ones1")
            nc.vector.memset(ones1[:], 1.0)

            idxA = pp.tile([P, NBLK * TA * 8], I16, tag="idxA")
            idxB = pp.tile([P, NBLK * TB * 8], I16, tag="idxB")
            cnts = pp.tile([1, cfg.nsub * NBLK], I32, tag="cnts")
            dstloc = pp.tile([P, TTOT], F32, tag="dstloc")
            nc.sync.dma_start(out=idxA[:], in_=pr["idxA"][:])
            nc.sync.dma_start(out=idxB[:], in_=pr["idxB"][:])
            nc.sync.dma_start(out=cnts[:], in_=pr["cnts"][:])
            nc.sync.dma_start(out=dstloc[:], in_=pr["dstloc"][:])

            wcat_sb = {}
            for i in (1, 2):
                w = pp.tile([FD + 1, CW], F32, tag=f"wcat{i}")
                nc.sync.dma_start(out=w[:], in_=pr[f"wcat{i}"][:])
                wcat_sb[i] = w
            wnj_sb, attn_sb, wfij_sb = [], [], []
            for i in range(3):
                w = pp.tile([FD + 1, FD], F32, tag=f"wnj{i}")
                nc.sync.dma_start(out=w[:], in_=pr[f"wnj{i}"][:])
                wnj_sb.append(w)
                a = pp.tile([P, FD], F32, tag=f"attn{i}")
                nc.sync.dma_start(out=a[:], in_=pr[f"attn{i}"][:])
                attn_sb.append(a)
                cdim = c.odf if i == 0 else FD
                w2 = pp.tile([cdim, FD], BF16, tag=f"wfij{i}")
                nc.sync.dma_start(out=w2[:], in_=pr[f"wfij{i}"][:])
                wfij_sb.append(w2)

            nh_sliceT = pp.tile([FD + 1, R], F32, tag="nh_sliceT")
            nc.sync.dma_start(out=nh_sliceT[:], in_=pr["nh0T"][:])

            # gather destination tiles: manually double-buffered and zeroed
            # once, so pad slots (skipped by the gather) always hold finite
            # values for the zero one-hot columns to nullify.
            G0 = pp.tile([P, T * CW], F32, tag="G0")
            G1 = pp.tile([P, T * CW], F32, tag="G1")
            G_bufs = [G0, G1]
            nc.vector.memset(G_bufs[0][:], 0.0)
            nc.vector.memset(G_bufs[1][:], 0.0)

            ra = nc.gpsimd.alloc_register("ra")
            rb = nc.gpsimd.alloc_register("rb")

            def ap(t, offset, pattern):
                v = t[:]
                return bass.AP(v.tensor, v.offset + offset, pattern)

            def tables(l):
                """catL = nh_sliceT @ wcat_l; AllGather into catGA/catGB."""
                wc = wcat_sb[l]
                with (
                    tc.tile_pool(name="tabw", bufs=2) as wp,
                    tc.tile_pool(name="tabp", bufs=2, space="PSUM") as qp,
                ):
                    for nb in range(NBLK):
                        pt = qp.tile([P, CW], F32, tag="ptab")
                        nc.tensor.matmul(
                            out=pt[:],
                            lhsT=nh_sliceT[:, nb * P : (nb + 1) * P],
                            rhs=wc[:],
                            start=True, stop=True,
                        )
                        cs = wp.tile([P, CW], F32, tag="catsb")
                        nc.scalar.activation(out=cs[:], in_=pt[:], func=AF.Copy)
                        nc.sync.dma_start(
                            out=catL[nb * P : (nb + 1) * P, :], in_=cs[:]
                        )
                nc.gpsimd.collective_compute(
                    "AllGather", ALU.bypass, replica_groups=rg,
                    ins=[catL[0 : CROWS[0], :]], outs=[catG[l][0][:]],
                )
                nc.gpsimd.collective_compute(
                    "AllGather", ALU.bypass, replica_groups=rg,
                    ins=[catL[CROWS[0] : R, :]], outs=[catG[l][1][:]],
                )

            def edge_pass(l):
                cdim = c.odf if l == 0 else FD
                wf = wfij_sb[l]
                attn = attn_sb[l]
                wnj = wnj_sb[l]
                catA_t = pr["catA0"] if l == 0 else catG[l][0]
                catB_t = pr["catB0"] if l == 0 else catG[l][1]
                ef_dst = efA if l == 0 else (efB if l == 1 else None)
                ef_rows = efA if l == 1 else efB  # row-major source (l>=1)

                with (
                    tc.tile_pool(name="pb", bufs=2) as pb,
                    tc.tile_pool(name="pc", bufs=3) as pc,
                    tc.tile_pool(name="qrep", bufs=2, space="PSUM") as qrep,
                    tc.tile_pool(name="qfp", bufs=2, space="PSUM") as qfp,
                    tc.tile_pool(name="qps", bufs=2, space="PSUM") as qps,
                    tc.tile_pool(name="qnh", bufs=1, space="PSUM") as qnh,
                ):
                    for b in range(NBLK):
                        # ---- gathers: G = [ni|ns] rows of this block ----
                        G = G_bufs[b % 2]
                        ci = b * cfg.nsub
                        for gi, (idxt, Tbase, Toff, subs, cat_t) in enumerate(
                            [
                                (idxA, TA, 0, cfg.subsA, catA_t),
                                (idxB, TB, TA, cfg.subsB, catB_t),
                            ]
                        ):
                            for (t0, tn) in subs:
                                nc.gpsimd.reg_load(ra, cnts[0:1, ci : ci + 1])
                                ci += 1
                                nc.gpsimd.dma_gather(
                                    out_ap=ap(
                                        G, (Toff + t0) * CW,
                                        [G[:].ap[0], [CW, tn], [1, CW]],
                                    ),
                                    in_ap=cat_t[:, :],
                                    idxs_ap=idxt[
                                        :,
                                        b * Tbase * 8 + t0 * 8 :
                                        b * Tbase * 8 + (t0 + tn) * 8,
                                    ],
                                    num_idxs=tn * P, num_idxs_reg=ra,
                                    elem_size=CW,
                                )

                        if stage <= 1:
                            htile = pb.tile([P, FD], F32, tag="htile")
                            nc.vector.tensor_copy(out=htile[:], in_=G[:, 0:FD])
                            nc.sync.dma_start(
                                out=out3[b * P : (b + 1) * P, :], in_=htile[:]
                            )
                            continue

                        # ---- block-local tables ----
                        dT = pb.tile([1, T * P], BF16, tag="dT")
                        nc.sync.dma_start(
                            out=dT[:], in_=pr["dstlocT"][:, b * T * P : (b + 1) * T * P]
                        )
                        njp = qnh.tile([P, FD], F32, tag="njp")
                        nc.tensor.matmul(
                            out=njp[:], lhsT=nh_sliceT[:, b * P : (b + 1) * P],
                            rhs=wnj[:], start=True, stop=True,
                        )
                        njb = pb.tile([P, FD], BF16, tag="njb")
                        nc.scalar.activation(out=njb[:], in_=njp[:], func=AF.Copy)

                        efc = pc.tile([cdim, T * P], BF16, tag="efc")
                        if l == 0:
                            nc.sync.dma_start(
                                out=efc[:],
                                in_=pr["ef0T"][:, b * T * P : (b + 1) * T * P],
                            )
                        else:
                            nc.sync.dma_start_transpose(
                                efc[:],
                                ef_rows[b * T * P : (b + 1) * T * P, 0:FD],
                            )

                        oh = pb.tile([P, T * P], BF16, tag="oh")
                        ohT = pb.tile([P, T * P], BF16, tag="ohT")
                        t2 = pb.tile([P, T * FD], F32, tag="t2")

                        for c0 in range(0, T, c.tc):
                            tcn = min(c.tc, T - c0)
                            # one-hots for the chunk (edge-major on DVE;
                            # node-major via PE broadcast of dT then DVE)
                            nc.vector.tensor_tensor(
                                out=ap(oh, c0 * P, [oh[:].ap[0], [P, tcn], [1, P]]),
                                in0=ap(dstloc, b * T + c0, [dstloc[:].ap[0], [1, tcn], [0, P]]),
                                in1=ap(iota_f, 0, [iota_f[:].ap[0], [0, tcn], [1, P]]),
                                op=ALU.is_equal,
                            )
                            dRep = qrep.tile([P, c.tc * P], F32, tag="dRep")
                            nc.tensor.matmul(
                                out=dRep[:, 0 : tcn * P],
                                lhsT=ones1[:],
                                rhs=dT[:, c0 * P : (c0 + tcn) * P],
                                start=True, stop=True,
                            )
                            nc.vector.tensor_tensor(
                                out=ap(ohT, c0 * P, [ohT[:].ap[0], [P, tcn], [1, P]]),
                                in0=ap(iota_c, 0, [iota_c[:].ap[0], [0, tcn], [0, P]]),
                                in1=ap(dRep, 0, [dRep[:].ap[0], [P, tcn], [1, P]]),
                                op=ALU.is_equal,
                            )
                            fp = qfp.tile([P, c.tc * FD], F32, tag="fp")
                            for t in range(tcn):
                                nc.tensor.matmul(
                                    out=fp[:, t * FD : (t + 1) * FD],
                                    lhsT=ohT[:, (c0 + t) * P : (c0 + t + 1) * P],
                                    rhs=njb[:],
                                    start=True, stop=False,
                                    skip_group_check=True,
                                )
                                nc.tensor.matmul(
                                    out=fp[:, t * FD : (t + 1) * FD],
                                    lhsT=efc[:, (c0 + t) * P : (c0 + t + 1) * P],
                                    rhs=wf[:],
                                    start=False, stop=True,
                                    skip_group_check=True,
                                )
                            # t2 = fp + ni
                            nc.vector.tensor_tensor(
                                out=ap(t2, c0 * FD, [t2[:].ap[0], [FD, tcn], [1, FD]]),
                                in0=fp[:, 0 : tcn * FD].rearrange(
                                    "p (t f) -> p t f", t=tcn
                                ),
                                in1=ap(G, c0 * CW, [G[:].ap[0], [CW, tcn], [1, FD]]),
                                op=ALU.add,
                            )

                        if stage <= 2:
                            htile = pb.tile([P, FD], F32, tag="htile")
                            nc.vector.tensor_copy(out=htile[:], in_=t2[:, 0:FD])
                            nc.sync.dma_start(
                                out=out3[b * P : (b + 1) * P, :], in_=htile[:]
                            )
                            continue

                        # ---- leaky + logits + softmax numerator ----
                        fl = pb.tile([P, T * FD], F32, tag="fl")
                        nc.vector.scalar_tensor_tensor(
                            out=fl[:], in0=t2[:], scalar=0.01, in1=t2[:],
                            op0=ALU.mult, op1=ALU.max,
                        )
                        # logit products overwrite G's ni columns (dead after t2)
                        nc.gpsimd.tensor_tensor(
                            out=ap(G, 0, [G[:].ap[0], [CW, T], [1, FD]]),
                            in0=fl[:].rearrange("p (t f) -> p t f", t=T),
                            in1=ap(attn, 0, [attn[:].ap[0], [0, T], [1, FD]]),
                            op=ALU.mult,
                        )
                        eL = pb.tile([P, T * H], F32, tag="eL")
                        nc.vector.tensor_reduce(
                            out=eL[:].rearrange("p (t h) -> p t h", t=T),
                            in_=ap(G, 0, [G[:].ap[0], [CW, T], [HE, H], [1, HE]]),
                            axis=mybir.AxisListType.X, op=ALU.add,
                        )
                        nc.vector.tensor_scalar(
                            out=eL[:], in0=eL[:], scalar1=60.0, scalar2=None,
                            op0=ALU.min,
                        )
                        nc.scalar.activation(out=eL[:], in_=eL[:], func=AF.Exp)
                        if dbg and b == 0 and l == 0:
                            nc.sync.dma_start(out=dbg_t["d_G"][:], in_=G[:])
                            nc.sync.dma_start(out=dbg_t["d_eL"][:], in_=eL[:])
                            nc.sync.dma_start(out=dbg_t["d_t2"][:], in_=t2[:])

                        if stage <= 3:
                            htile = pb.tile([P, FD], F32, tag="htile")
                            nc.vector.tensor_copy(out=htile[:], in_=fl[:, 0:FD])
                            nc.sync.dma_start(
                                out=out3[b * P : (b + 1) * P, :], in_=htile[:]
                            )
                            continue

                        # ---- messages (bf16) ----
                        me = pb.tile([P, T * (FD + H)], BF16, tag="me")
                        nc.gpsimd.tensor_tensor(
                            out=ap(me, 0, [me[:].ap[0], [FD + H, T], [HE, H], [1, HE]]),
                            in0=ap(G, FD, [G[:].ap[0], [CW, T], [HE, H], [1, HE]]),
                            in1=ap(eL, 0, [eL[:].ap[0], [H, T], [1, H], [0, HE]]),
                            op=ALU.mult,
                        )
                        nc.vector.tensor_copy(
                            out=ap(me, FD, [me[:].ap[0], [FD + H, T], [1, H]]),
                            in_=eL[:].rearrange("p (t h) -> p t h", t=T),
                        )

                        # ---- scatter [h|s] ----
                        ps = qps.tile([P, FD + H], F32, tag="ps")
                        for t in range(T):
                            nc.tensor.matmul(
                                out=ps[:],
                                lhsT=oh[:, t * P : (t + 1) * P],
                                rhs=me[:, t * (FD + H) : (t + 1) * (FD + H)],
                                start=(t == 0), stop=(t == T - 1),
                                skip_group_check=True,
                            )

                        if stage <= 4:
                            htile = pb.tile([P, FD], F32, tag="htile")
                            nc.scalar.activation(
                                out=htile[:], in_=ps[:, 0:FD], func=AF.Copy
                            )
                            nc.sync.dma_start(
                                out=out3[b * P : (b + 1) * P, :], in_=htile[:]
                            )
                            continue

                        # ---- store edge features for next layer ----
                        if stage > 5 and ef_dst is not None:
                            fs = pb.tile([P, T * FD], BF16, tag="fs")
                            nc.scalar.activation(out=fs[:], in_=t2[:], func=AF.Relu)
                            efout = bass.AP(
                                ef_dst[:].tensor,
                                ef_dst[:].offset + b * T * P * FD,
                                [[FD, P], [P * FD, T], [1, FD]],
                            )
                            nc.sync.dma_start(
                                out=efout,
                                in_=fs[:].rearrange("p (t f) -> p t f", t=T),
                            )

                        # ---- h = ps/s ----
                        sp = pb.tile([P, H], F32, tag="sp")
                        nc.vector.tensor_scalar_add(
                            out=sp[:], in0=ps[:, FD : FD + H], scalar1=EPS
                        )
                        rcp = pb.tile([P, H], F32, tag="rcp")
                        nc.vector.reciprocal_approx_fast(out=rcp[:], in_=sp[:])
                        rv = ap(rcp, 0, [rcp[:].ap[0], [1, H], [0, HE]])
                        htile = pb.tile([P, FD], F32, tag="htile")
                        if l < 2:
                            hr = pb.tile([P, FD], F32, tag="hr")
                            nc.scalar.activation(
                                out=hr[:], in_=ps[:, 0:FD], func=AF.Relu
                            )
                            nc.vector.tensor_tensor(
                                out=htile[:].rearrange("p (h d) -> p h d", h=H),
                                in0=hr[:].rearrange("p (h d) -> p h d", h=H),
                                in1=rv, op=ALU.mult,
                            )
                            hT = qnh.tile([FD, P], F32, tag="hT")
                            nc.tensor.transpose(
                                out=hT[:], in_=htile[:], identity=ident[:]
                            )
                            nc.scalar.activation(
                                out=nh_sliceT[0:FD, b * P : (b + 1) * P],
                                in_=hT[:], func=AF.Copy,
                            )
                        else:
                            nc.vector.tensor_tensor(
                                out=htile[:].rearrange("p (h d) -> p h d", h=H),
                                in0=ps[:, 0:FD].rearrange("p (h d) -> p h d", h=H),
                                in1=rv, op=ALU.mult,
                            )
                            nc.sync.dma_start(
                                out=out3[b * P : (b + 1) * P, :], in_=htile[:]
                            )

            edge_pass(0)
            if nlayers > 1:
                tables(1)
                edge_pass(1)
            if nlayers > 2:
                tables(2)
                edge_pass(2)
            if dbg:
                nc.sync.dma_start(out=dbg_t["d_nh"][:], in_=nh_sliceT[:])

    nc.compile()
    return nc


_CACHE = {}


def get_program(cfg):
    key = (cfg.TA, cfg.TB, cfg.odf, cfg.tc)
    if key not in _CACHE:
        _CACHE[key] = build_program(cfg)
    return _CACHE[key]


def run(inputs, trace=False):
    from concourse.bass_utils import run_bass_kernel_spmd

    percore, cfg = host_prep(inputs)
    nc = get_program(cfg)
    res = run_bass_kernel_spmd(nc, percore, list(range(NDEV)), trace=trace)
    outs = [res.results[i]["out3"] for i in range(NDEV)]
    full = np.concatenate(outs, axis=0)  # [NPAD, 96]
    return full, res


def kernel(**inputs) -> np.ndarray:
    full, _ = run(inputs)
    idxs = np.asarray(inputs["idxs"]).astype(np.int64)
    return np.ascontiguousarray(full[idxs]).astype(np.float32)


# revision 54
# speedup vs baseline: 4.7372x; 4.7372x over previous
"""Trainium2 Bass kernel for 3-layer EGAT message passing (nn_COUNTYOD).

Strategy (8 cores, edge parallelism by dst range):
  - Host: sort edges by dst; device d owns dst nodes [d*6272, (d+1)*6272);
    edges grouped into 49 blocks of 128 dst nodes. Within a block, edges are
    split into two groups by src chunk (first 24 / last 25 blocks of the
    src's owner device), each padded to a tile multiple (TA/TB tiles).
  - Gather tables catA/catB hold rows [nh@Wni + b | nh@Wns] (192 f32 =
    768B, %256==0) in chunk-major order, so a whole block's src rows are
    fetched with TWO dma_gather instructions (int16 idxs, -1 suffix pad,
    per-device counts via reg_load) instead of per-tile indirect DMAs.
  - nj[dst] is block-local: njb = nh_sliceT[block] @ Wnj once per block,
    then selected per tile with a transposed one-hot matmul on PE.
  - Single fused pass per block: f_pre (PSUM) -> leaky (Act Prelu) ->
    logits (mul+grouped reduce) -> ex=exp(min(e,60)) -> msg=ns*ex (bf16)
    -> one-hot scatter matmul accumulating [h | s] in PSUM -> h = ps/s.
  - Edge features stored row-major bf16 [E,96]; next layer loads them
    transposed via dma_start_transpose. All per-tile matmuls in bf16.
  - Layer-0 tables are host-precomputed params (no collective); layers 1-2
    AllGather the locally computed 192-col table in two chunks.
"""

import sys
import numpy as np

for _p in ("/opt/trn_rl_repo",):
    if _p not in sys.path:
        sys.path.insert(0, _p)

import ml_dtypes  # noqa: E402
import concourse.bass as bass  # noqa: E402
import concourse.bacc as bacc  # noqa: E402
import concourse.mybir as mybir  # noqa: E402
import concourse.tile as tile  # noqa: E402
from concourse.masks import make_identity  # noqa: E402

F32 = mybir.dt.float32
BF16 = mybir.dt.bfloat16
I32 = mybir.dt.int32
I16 = mybir.dt.int16
AF = mybir.ActivationFunctionType
ALU = mybir.AluOpType
BNP = ml_dtypes.bfloat16

P = 128
FD = 96           # H*HE
CW = 192          # gather row width [ni|ns]
H = 3
HE = 32
EPS = 1e-20
N_REAL = 50000
NDEV = 8
NBLK = 49
R = NBLK * P      # 6272 nodes per device
NPAD = NDEV * R
CBLK = (24, 25)   # blocks per src chunk (within device)
CROWS = (CBLK[0] * P, CBLK[1] * P)          # 3072, 3200
CTOT = (NDEV * CROWS[0], NDEV * CROWS[1])   # 24576, 25600 rows per table


GSUB = 8          # max tiles (1024 idxs) per dma_gather (SWDGE ring limit)


def _subs(Tn):
    """Split Tn tiles into sub-gather spans of <= GSUB tiles."""
    out = []
    t0 = 0
    while t0 < Tn:
        out.append((t0, min(GSUB, Tn - t0)))
        t0 += GSUB
    return out


class Cfg:
    def __init__(self, TA, TB, odf, tc=4):
        self.TA = TA
        self.TB = TB
        self.T = TA + TB
        self.odf = odf
        self.tc = tc
        self.ttot = NBLK * self.T
        self.epd = self.ttot * P
        self.subsA = _subs(TA)
        self.subsB = _subs(TB)
        self.nsub = len(self.subsA) + len(self.subsB)


def _wrap16(v):
    """idx list -> [16, n/16] layout (value i at [i%16, i//16])."""
    return np.ascontiguousarray(v.reshape(-1, 16).T)


def host_prep(inputs, cfg=None):
    src = np.asarray(inputs["src"]).astype(np.int64)
    dst = np.asarray(inputs["dst"]).astype(np.int64)
    E = src.shape[0]

    order = np.argsort(dst, kind="stable")
    ssrc, sdst = src[order], dst[order]
    ef0 = np.asarray(inputs["countyodfeats"]).astype(np.float32)[order]
    odf = ef0.shape[1]

    # chunk id + table row for each (sorted) edge's src
    d_s = ssrc // R
    r_s = ssrc % R
    k_s = (r_s >= CROWS[0]).astype(np.int64)
    row_s = np.where(k_s == 0, d_s * CROWS[0] + r_s,
                     d_s * CROWS[1] + (r_s - CROWS[0]))

    blk = sdst // P
    blkcnt = np.bincount(blk, minlength=NPAD // P)
    starts = np.zeros(NPAD // P + 1, np.int64)
    np.cumsum(blkcnt, out=starts[1:])

    # per (dev, blk) group sizes to fix TA/TB
    nlo = np.zeros((NDEV, NBLK), np.int64)
    nhi = np.zeros((NDEV, NBLK), np.int64)
    for d in range(NDEV):
        for b in range(NBLK):
            g = d * NBLK + b
            ks = k_s[starts[g]:starts[g + 1]]
            nlo[d, b] = int((ks == 0).sum())
            nhi[d, b] = int((ks == 1).sum())
    TA = max(1, int(np.ceil(nlo.max() / P)))
    TB = max(1, int(np.ceil(nhi.max() / P)))
    if cfg is None:
        cfg = Cfg(TA, TB, odf)
    else:
        assert cfg.TA >= TA and cfg.TB >= TB
    T, TP = cfg.T, cfg.T * P

    def subcnt(n, t0, tn):
        """valid count within sub-gather span [t0*P, t0*P+tn*P) of a
        region with n valid prefix entries; >=1 (a forced idx 0 covers 0)."""
        return max(1, min(n - t0 * P, tn * P)) if n > t0 * P else 1

    percore = []
    for d in range(NDEV):
        idxA = np.full((NBLK, cfg.TA * P), -1, np.int64)
        idxB = np.full((NBLK, cfg.TB * P), -1, np.int64)
        cnts = np.zeros((1, cfg.nsub * NBLK), np.int32)
        dloc = np.full((NBLK, TP), -1.0, np.float32)
        efp = np.zeros((NBLK, TP, odf), np.float32)
        for b in range(NBLK):
            g = d * NBLK + b
            s0, s1 = starts[g], starts[g + 1]
            ks = k_s[s0:s1]
            lo = np.nonzero(ks == 0)[0]
            hi = np.nonzero(ks == 1)[0]
            na, nb = len(lo), len(hi)
            idxA[b, :na] = row_s[s0:s1][lo]
            idxB[b, :nb] = row_s[s0:s1][hi]
            ci = b * cfg.nsub
            for (t0, tn) in cfg.subsA:
                cnts[0, ci] = subcnt(na, t0, tn)
                if na <= t0 * P:
                    idxA[b, t0 * P] = 0
                ci += 1
            for (t0, tn) in cfg.subsB:
                cnts[0, ci] = subcnt(nb, t0, tn)
                if nb <= t0 * P:
                    idxB[b, t0 * P] = 0
                ci += 1
            dloc[b, :na] = (sdst[s0:s1][lo] - g * P).astype(np.float32)
            dloc[b, cfg.TA * P : cfg.TA * P + nb] = (
                sdst[s0:s1][hi] - g * P
            ).astype(np.float32)
            efp[b, :na] = ef0[s0:s1][lo]
            efp[b, cfg.TA * P : cfg.TA * P + nb] = ef0[s0:s1][hi]
        ia = np.concatenate([_wrap16(idxA[b]) for b in range(NBLK)], axis=1)
        ib = np.concatenate([_wrap16(idxB[b]) for b in range(NBLK)], axis=1)
        dstloc = dloc.reshape(-1, P).T            # [128, ttot]
        dstlocT = dloc.reshape(1, -1)             # [1, epd]
        percore.append(
            dict(
                idxA=np.ascontiguousarray(np.tile(ia, (8, 1)).astype(np.int16)),
                idxB=np.ascontiguousarray(np.tile(ib, (8, 1)).astype(np.int16)),
                cnts=cnts,
                dstloc=np.ascontiguousarray(dstloc),
                dstlocT=np.ascontiguousarray(dstlocT.astype(BNP)),
                ef0T=np.ascontiguousarray(
                    efp.reshape(-1, odf).T.astype(BNP)
                ),
            )
        )

    g = lambda k: np.asarray(inputs[k]).astype(np.float32)
    nh0 = g("ndata_h")
    nh0p = np.zeros((NPAD, FD), np.float32)
    nh0p[:N_REAL] = nh0[:N_REAL]
    for d in range(NDEV):
        sl = nh0p[d * R : (d + 1) * R]
        percore[d]["nh0T"] = np.ascontiguousarray(
            np.concatenate([sl.T, np.ones((1, R), np.float32)], axis=0)
        )

    # host-precomputed layer-0 gather tables (chunk-major row order)
    ni0 = nh0p @ g("Wni0") + g("bias0")[None, :]
    ns0 = nh0p @ g("Wns0")
    cat0 = np.concatenate([ni0, ns0], axis=1).astype(np.float32)  # [NPAD,192]
    c4 = cat0.reshape(NDEV, R, CW)
    catA0 = np.ascontiguousarray(c4[:, : CROWS[0]].reshape(CTOT[0], CW))
    catB0 = np.ascontiguousarray(c4[:, CROWS[0] :].reshape(CTOT[1], CW))

    def wcat(Wni, Wns, bias):
        w = np.concatenate([Wni, Wns], axis=1)
        b = np.concatenate([bias, np.zeros(FD, np.float32)])[None, :]
        return np.ascontiguousarray(
            np.concatenate([w, b], axis=0).astype(np.float32)
        )

    def wnj(W):
        return np.ascontiguousarray(
            np.concatenate([W, np.zeros((1, FD), np.float32)], axis=0)
        )

    weights = dict(
        catA0=catA0,
        catB0=catB0,
        wcat1=wcat(g("Wni")[0], g("Wns")[0], g("bias")[0]),
        wcat2=wcat(g("Wni")[1], g("Wns")[1], g("bias")[1]),
        wnj0=wnj(g("Wnj0")),
        wnj1=wnj(g("Wnj")[0]),
        wnj2=wnj(g("Wnj")[1]),
        wfij0=np.ascontiguousarray(g("Wfij0").astype(BNP)),
        # layers 1-2 contract over the padded 128-row ef layout
        wfij1=np.ascontiguousarray(
            np.concatenate(
                [g("Wfij")[0], np.zeros((P - FD, FD), np.float32)], axis=0
            ).astype(BNP)
        ),
        wfij2=np.ascontiguousarray(
            np.concatenate(
                [g("Wfij")[1], np.zeros((P - FD, FD), np.float32)], axis=0
            ).astype(BNP)
        ),
        attn0=np.ascontiguousarray(np.repeat(g("attn0").reshape(1, FD), P, 0)),
        attn1=np.ascontiguousarray(
            np.repeat(g("attn").reshape(2, FD)[0:1], P, 0)
        ),
        attn2=np.ascontiguousarray(
            np.repeat(g("attn").reshape(2, FD)[1:2], P, 0)
        ),
    )
    for d in range(NDEV):
        percore[d].update(weights)
    return percore, cfg


def build_program(cfg, nlayers=3, dbg=False, stage=99):
    nc = bacc.Bacc("TRN2", target_bir_lowering=False, debug=False)
    c = cfg
    T, TA, TB, TTOT, EPD = c.T, c.TA, c.TB, c.ttot, c.epd

    # ---- I/O ----
    pr = {}
    pr["nh0T"] = nc.declare_dram_parameter("nh0T", [FD + 1, R], F32, isOutput=False)
    pr["ef0T"] = nc.declare_dram_parameter("ef0T", [c.odf, EPD], BF16, isOutput=False)
    pr["idxA"] = nc.declare_dram_parameter("idxA", [P, NBLK * TA * 8], I16, isOutput=False)
    pr["idxB"] = nc.declare_dram_parameter("idxB", [P, NBLK * TB * 8], I16, isOutput=False)
    pr["cnts"] = nc.declare_dram_parameter("cnts", [1, cfg.nsub * NBLK], I32, isOutput=False)
    pr["dstloc"] = nc.declare_dram_parameter("dstloc", [P, TTOT], F32, isOutput=False)
    pr["dstlocT"] = nc.declare_dram_parameter("dstlocT", [1, EPD], BF16, isOutput=False)
    pr["catA0"] = nc.declare_dram_parameter("catA0", [CTOT[0], CW], F32, isOutput=False)
    pr["catB0"] = nc.declare_dram_parameter("catB0", [CTOT[1], CW], F32, isOutput=False)
    for i in (1, 2):
        pr[f"wcat{i}"] = nc.declare_dram_parameter(f"wcat{i}", [FD + 1, CW], F32, isOutput=False)
    for i in range(3):
        pr[f"wnj{i}"] = nc.declare_dram_parameter(f"wnj{i}", [FD + 1, FD], F32, isOutput=False)
        pr[f"attn{i}"] = nc.declare_dram_parameter(f"attn{i}", [P, FD], F32, isOutput=False)
    pr["wfij0"] = nc.declare_dram_parameter("wfij0", [c.odf, FD], BF16, isOutput=False)
    pr["wfij1"] = nc.declare_dram_parameter("wfij1", [P, FD], BF16, isOutput=False)
    pr["wfij2"] = nc.declare_dram_parameter("wfij2", [P, FD], BF16, isOutput=False)
    out3 = nc.declare_dram_parameter("out3", [R, FD], F32, isOutput=True)
    dbg_t = {}
    if dbg:
        dbg_t["d_nh"] = nc.declare_dram_parameter("d_nh", [FD + 1, R], F32, isOutput=True)
        dbg_t["d_G"] = nc.declare_dram_parameter("d_G", [P, cfg.T * CW], F32, isOutput=True)
        dbg_t["d_eL"] = nc.declare_dram_parameter("d_eL", [P, cfg.T * H], F32, isOutput=True)
        dbg_t["d_t2"] = nc.declare_dram_parameter("d_t2", [P, cfg.T * FD], F32, isOutput=True)

    # ---- internal DRAM ----
    efA = nc.dram_tensor("efA", [EPD, P], BF16)
    efB = nc.dram_tensor("efB", [EPD, P], BF16)
    catL = {
        1: nc.dram_tensor("catL1", [R, CW], F32),
        2: nc.dram_tensor("catL2", [R, CW], F32),
    }
    catG = {
        1: (
            nc.dram_tensor("catGA1", [CTOT[0], CW], F32, addr_space="Shared"),
            nc.dram_tensor("catGB1", [CTOT[1], CW], F32, addr_space="Shared"),
        ),
        2: (
            nc.dram_tensor("catGA2", [CTOT[0], CW], F32, addr_space="Shared"),
            nc.dram_tensor("catGB2", [CTOT[1], CW], F32, addr_space="Shared"),
        ),
    }

    rg = [list(range(NDEV))]

    with tile.TileContext(nc) as tc:
        with tc.tile_pool(name="persist", bufs=1) as pp:
            ident = pp.tile([P, P], F32, tag="ident")
            make_identity(nc, ident[:])
            iota_i = pp.tile([P, P], I32, tag="iota_i")
            nc.gpsimd.iota(iota_i[:], pattern=[[1, P]], base=0, channel_multiplier=0)
            iota_f = pp.tile([P, P], F32, tag="iota_f")
            nc.vector.tensor_copy(out=iota_f[:], in_=iota_i[:])
            iota_ci = pp.tile([P, 1], I32, tag="iota_ci")
            nc.gpsimd.iota(iota_ci[:], pattern=[[0, 1]], base=0, channel_multiplier=1)
            iota_c = pp.tile([P, 1], F32, tag="iota_c")
            nc.vector.tensor_copy(out=iota_c[:], in_=iota_ci[:])
            ones1 = pp.tile([1, P], BF16, tag="ones1")
            nc.vector.memset(ones1[:], 1.0)

            idxA = pp.tile([P, NBLK * TA * 8], I16, tag="idxA")
            idxB = pp.tile([P, NBLK * TB * 8], I16, tag="idxB")
            cnts = pp.tile([1, cfg.nsub * NBLK], I32, tag="cnts")
            dstloc = pp.tile([P, TTOT], F32, tag="dstloc")
            nc.sync.dma_start(out=idxA[:], in_=pr["idxA"][:])
            nc.sync.dma_start(out=idxB[:], in_=pr["idxB"][:])
            nc.sync.dma_start(out=cnts[:], in_=pr["cnts"][:])
            nc.sync.dma_start(out=dstloc[:], in_=pr["dstloc"][:])

            wcat_sb = {}
            for i in (1, 2):
                w = pp.tile([FD + 1, CW], F32, tag=f"wcat{i}")
                nc.sync.dma_start(out=w[:], in_=pr[f"wcat{i}"][:])
                wcat_sb[i] = w
            wnj_sb, attn_sb, wfij_sb = [], [], []
            for i in range(3):
                w = pp.tile([FD + 1, FD], F32, tag=f"wnj{i}")
                nc.sync.dma_start(out=w[:], in_=pr[f"wnj{i}"][:])
                wnj_sb.append(w)
                a = pp.tile([P, FD], F32, tag=f"attn{i}")
                nc.sync.dma_start(out=a[:], in_=pr[f"attn{i}"][:])
                attn_sb.append(a)
                cdim = c.odf if i == 0 else P
                w2 = pp.tile([cdim, FD], BF16, tag=f"wfij{i}")
                nc.sync.dma_start(out=w2[:], in_=pr[f"wfij{i}"][:])
                wfij_sb.append(w2)

            nh_sliceT = pp.tile([FD + 1, R], F32, tag="nh_sliceT")
            nc.sync.dma_start(out=nh_sliceT[:], in_=pr["nh0T"][:])

            # gather destination tiles: manually double-buffered and zeroed
            # once, so pad slots (skipped by the gather) always hold finite
            # values for the zero one-hot columns to nullify.
            G0 = pp.tile([P, T * CW], F32, tag="G0")
            G1 = pp.tile([P, T * CW], F32, tag="G1")
            G_bufs = [G0, G1]
            nc.vector.memset(G_bufs[0][:], 0.0)
            nc.vector.memset(G_bufs[1][:], 0.0)
            # stored edge features, padded to 128 cols; pads stay zero
            fs0 = pp.tile([P, T * P], BF16, tag="fs0")
            fs1 = pp.tile([P, T * P], BF16, tag="fs1")
            fs_bufs = [fs0, fs1]
            nc.vector.memset(fs_bufs[0][:], 0.0)
            nc.vector.memset(fs_bufs[1][:], 0.0)

            ra = nc.gpsimd.alloc_register("ra")
            rb = nc.gpsimd.alloc_register("rb")

            def ap(t, offset, pattern):
                v = t[:]
                return bass.AP(v.tensor, v.offset + offset, pattern)

            def edge_pass(l):
                cdim = c.odf if l == 0 else P
                wf = wfij_sb[l]
                attn = attn_sb[l]
                wnj = wnj_sb[l]
                catA_t = pr["catA0"] if l == 0 else catG[l][0]
                catB_t = pr["catB0"] if l == 0 else catG[l][1]
                ef_dst = efA if l == 0 else (efB if l == 1 else None)
                ef_rows = efA if l == 1 else efB  # row-major source (l>=1)
                prep = (l < 2) and (nlayers > l + 1) and stage > 5

                with (
                    tc.tile_pool(name="pb", bufs=2) as pb,
                    tc.tile_pool(name="pc", bufs=3) as pc,
                    tc.tile_pool(name="qrep", bufs=1, space="PSUM") as qrep,
                    tc.tile_pool(name="qfp", bufs=2, space="PSUM") as qfp,
                    tc.tile_pool(name="qps", bufs=2, space="PSUM") as qps,
                    tc.tile_pool(name="qnh", bufs=1, space="PSUM") as qnh,
                ):
                    for b in range(NBLK):
                        # ---- gathers: G = [ni|ns] rows of this block ----
                        G = G_bufs[b % 2]
                        ci = b * cfg.nsub
                        for gi, (idxt, Tbase, Toff, subs, cat_t) in enumerate(
                            [
                                (idxA, TA, 0, cfg.subsA, catA_t),
                                (idxB, TB, TA, cfg.subsB, catB_t),
                            ]
                        ):
                            for (t0, tn) in subs:
                                nc.gpsimd.reg_load(ra, cnts[0:1, ci : ci + 1])
                                ci += 1
                                nc.gpsimd.dma_gather(
                                    out_ap=ap(
                                        G, (Toff + t0) * CW,
                                        [G[:].ap[0], [CW, tn], [1, CW]],
                                    ),
                                    in_ap=cat_t[:, :],
                                    idxs_ap=idxt[
                                        :,
                                        b * Tbase * 8 + t0 * 8 :
                                        b * Tbase * 8 + (t0 + tn) * 8,
                                    ],
                                    num_idxs=tn * P, num_idxs_reg=ra,
                                    elem_size=CW,
                                )

                        if stage <= 1:
                            htile = pb.tile([P, FD], F32, tag="htile")
                            nc.vector.tensor_copy(out=htile[:], in_=G[:, 0:FD])
                            nc.sync.dma_start(
                                out=out3[b * P : (b + 1) * P, :], in_=htile[:]
                            )
                            continue

                        # ---- block-local tables ----
                        dT = pb.tile([1, T * P], BF16, tag="dT")
                        nc.sync.dma_start(
                            out=dT[:], in_=pr["dstlocT"][:, b * T * P : (b + 1) * T * P]
                        )
                        njp = qnh.tile([P, FD], F32, tag="njp")
                        nc.tensor.matmul(
                            out=njp[:], lhsT=nh_sliceT[:, b * P : (b + 1) * P],
                            rhs=wnj[:], start=True, stop=True,
                        )
                        njb = pb.tile([P, FD], BF16, tag="njb")
                        nc.scalar.activation(out=njb[:], in_=njp[:], func=AF.Copy)

                        efc = pc.tile([cdim, T * P], BF16, tag="efc")
                        if l == 0:
                            nc.sync.dma_start(
                                out=efc[:],
                                in_=pr["ef0T"][:, b * T * P : (b + 1) * T * P],
                            )
                        else:
                            nc.sync.dma_start_transpose(
                                efc[:],
                                ef_rows[b * T * P : (b + 1) * T * P, :],
                            )

                        oh = pb.tile([P, T * P], BF16, tag="oh")
                        ohT = pb.tile([P, T * P], BF16, tag="ohT")
                        t2 = pb.tile([P, T * FD], F32, tag="t2")

                        for c0 in range(0, T, c.tc):
                            tcn = min(c.tc, T - c0)
                            # one-hots for the chunk (edge-major on DVE;
                            # node-major via PE broadcast of dT then DVE)
                            nc.vector.tensor_tensor(
                                out=ap(oh, c0 * P, [oh[:].ap[0], [P, tcn], [1, P]]),
                                in0=ap(dstloc, b * T + c0, [dstloc[:].ap[0], [1, tcn], [0, P]]),
                                in1=ap(iota_f, 0, [iota_f[:].ap[0], [0, tcn], [1, P]]),
                                op=ALU.is_equal,
                            )
                            dRep = qrep.tile([P, c.tc * P], F32, tag="dRep")
                            nc.tensor.matmul(
                                out=dRep[:, 0 : tcn * P],
                                lhsT=ones1[:],
                                rhs=dT[:, c0 * P : (c0 + tcn) * P],
                                start=True, stop=True,
                            )
                            nc.vector.tensor_tensor(
                                out=ap(ohT, c0 * P, [ohT[:].ap[0], [P, tcn], [1, P]]),
                                in0=ap(iota_c, 0, [iota_c[:].ap[0], [0, tcn], [0, P]]),
                                in1=ap(dRep, 0, [dRep[:].ap[0], [P, tcn], [1, P]]),
                                op=ALU.is_equal,
                            )
                            fp = qfp.tile([P, c.tc * FD], F32, tag="fp")
                            for t in range(tcn):
                                nc.tensor.matmul(
                                    out=fp[:, t * FD : (t + 1) * FD],
                                    lhsT=ohT[:, (c0 + t) * P : (c0 + t + 1) * P],
                                    rhs=njb[:],
                                    start=True, stop=False,
                                    skip_group_check=True,
                                )
                                nc.tensor.matmul(
                                    out=fp[:, t * FD : (t + 1) * FD],
                                    lhsT=efc[:, (c0 + t) * P : (c0 + t + 1) * P],
                                    rhs=wf[:],
                                    start=False, stop=True,
                                    skip_group_check=True,
                                )
                            # t2 = fp + ni
                            nc.vector.tensor_tensor(
                                out=ap(t2, c0 * FD, [t2[:].ap[0], [FD, tcn], [1, FD]]),
                                in0=fp[:, 0 : tcn * FD].rearrange(
                                    "p (t f) -> p t f", t=tcn
                                ),
                                in1=ap(G, c0 * CW, [G[:].ap[0], [CW, tcn], [1, FD]]),
                                op=ALU.add,
                            )

                        if stage <= 2:
                            htile = pb.tile([P, FD], F32, tag="htile")
                            nc.vector.tensor_copy(out=htile[:], in_=t2[:, 0:FD])
                            nc.sync.dma_start(
                                out=out3[b * P : (b + 1) * P, :], in_=htile[:]
                            )
                            continue

                        # ---- leaky + logits + softmax numerator ----
                        fl = pb.tile([P, T * FD], F32, tag="fl")
                        nc.vector.scalar_tensor_tensor(
                            out=fl[:], in0=t2[:], scalar=0.01, in1=t2[:],
                            op0=ALU.mult, op1=ALU.max,
                        )
                        # logit products overwrite G's ni columns (dead after t2)
                        nc.vector.tensor_tensor(
                            out=ap(G, 0, [G[:].ap[0], [CW, T], [1, FD]]),
                            in0=fl[:].rearrange("p (t f) -> p t f", t=T),
                            in1=ap(attn, 0, [attn[:].ap[0], [0, T], [1, FD]]),
                            op=ALU.mult,
                        )
                        eL = pb.tile([P, T * H], F32, tag="eL")
                        nc.vector.tensor_reduce(
                            out=eL[:].rearrange("p (t h) -> p t h", t=T),
                            in_=ap(G, 0, [G[:].ap[0], [CW, T], [HE, H], [1, HE]]),
                            axis=mybir.AxisListType.X, op=ALU.add,
                        )
                        nc.vector.tensor_scalar(
                            out=eL[:], in0=eL[:], scalar1=60.0, scalar2=None,
                            op0=ALU.min,
                        )
                        nc.scalar.activation(out=eL[:], in_=eL[:], func=AF.Exp)
                        if dbg and b == 0 and l == 0:
                            nc.sync.dma_start(out=dbg_t["d_G"][:], in_=G[:])
                            nc.sync.dma_start(out=dbg_t["d_eL"][:], in_=eL[:])
                            nc.sync.dma_start(out=dbg_t["d_t2"][:], in_=t2[:])

                        if stage <= 3:
                            htile = pb.tile([P, FD], F32, tag="htile")
                            nc.vector.tensor_copy(out=htile[:], in_=fl[:, 0:FD])
                            nc.sync.dma_start(
                                out=out3[b * P : (b + 1) * P, :], in_=htile[:]
                            )
                            continue

                        # ---- messages (bf16) ----
                        me = pb.tile([P, T * (FD + H)], BF16, tag="me")
                        nc.gpsimd.tensor_tensor(
                            out=ap(me, 0, [me[:].ap[0], [FD + H, T], [HE, H], [1, HE]]),
                            in0=ap(G, FD, [G[:].ap[0], [CW, T], [HE, H], [1, HE]]),
                            in1=ap(eL, 0, [eL[:].ap[0], [H, T], [1, H], [0, HE]]),
                            op=ALU.mult,
                        )
                        nc.scalar.activation(
                            out=ap(me, FD, [me[:].ap[0], [FD + H, T], [1, H]]),
                            in_=eL[:].rearrange("p (t h) -> p t h", t=T),
                            func=AF.Copy,
                        )

                        # ---- scatter [h|s] ----
                        ps = qps.tile([P, FD + H], F32, tag="ps")
                        for t in range(T):
                            nc.tensor.matmul(
                                out=ps[:],
                                lhsT=oh[:, t * P : (t + 1) * P],
                                rhs=me[:, t * (FD + H) : (t + 1) * (FD + H)],
                                start=(t == 0), stop=(t == T - 1),
                                skip_group_check=True,
                            )

                        if stage <= 4:
                            htile = pb.tile([P, FD], F32, tag="htile")
                            nc.scalar.activation(
                                out=htile[:], in_=ps[:, 0:FD], func=AF.Copy
                            )
                            nc.sync.dma_start(
                                out=out3[b * P : (b + 1) * P, :], in_=htile[:]
                            )
                            continue

                        # ---- store edge features for next layer ----
                        if stage > 5 and ef_dst is not None:
                            fs = fs_bufs[b % 2]
                            nc.scalar.activation(
                                out=ap(fs, 0, [fs[:].ap[0], [P, T], [1, FD]]),
                                in_=t2[:].rearrange("p (t f) -> p t f", t=T),
                                func=AF.Relu,
                            )
                            efout = bass.AP(
                                ef_dst[:].tensor,
                                ef_dst[:].offset + b * T * P * P,
                                [[P, P], [P * P, T], [1, P]],
                            )
                            nc.sync.dma_start(
                                out=efout,
                                in_=fs[:].rearrange("p (t f) -> p t f", t=T),
                            )

                        # ---- h = ps/s ----
                        sp = pb.tile([P, H], F32, tag="sp")
                        nc.vector.tensor_scalar_add(
                            out=sp[:], in0=ps[:, FD : FD + H], scalar1=EPS
                        )
                        rcp = pb.tile([P, H], F32, tag="rcp")
                        nc.vector.reciprocal_approx_fast(out=rcp[:], in_=sp[:])
                        rv = ap(rcp, 0, [rcp[:].ap[0], [1, H], [0, HE]])
                        htile = pb.tile([P, FD], F32, tag="htile")
                        if l < 2:
                            hr = pb.tile([P, FD], F32, tag="hr")
                            nc.scalar.activation(
                                out=hr[:], in_=ps[:, 0:FD], func=AF.Relu
                            )
                            nc.vector.tensor_tensor(
                                out=htile[:].rearrange("p (h d) -> p h d", h=H),
                                in0=hr[:].rearrange("p (h d) -> p h d", h=H),
                                in1=rv, op=ALU.mult,
                            )
                            hT = qnh.tile([FD, P], F32, tag="hT")
                            nc.tensor.transpose(
                                out=hT[:], in_=htile[:], identity=ident[:]
                            )
                            nc.scalar.activation(
                                out=nh_sliceT[0:FD, b * P : (b + 1) * P],
                                in_=hT[:], func=AF.Copy,
                            )
                            if prep:
                                # next layer's gather-table rows for this block
                                pt = qnh.tile([P, CW], F32, tag="pt")
                                nc.tensor.matmul(
                                    out=pt[:],
                                    lhsT=nh_sliceT[:, b * P : (b + 1) * P],
                                    rhs=wcat_sb[l + 1][:],
                                    start=True, stop=True,
                                )
                                cs = pb.tile([P, CW], F32, tag="cs")
                                nc.scalar.activation(
                                    out=cs[:], in_=pt[:], func=AF.Copy
                                )
                                nc.sync.dma_start(
                                    out=catL[l + 1][b * P : (b + 1) * P, :],
                                    in_=cs[:],
                                )
                                if b == CBLK[0] - 1:
                                    nc.gpsimd.collective_compute(
                                        "AllGather", ALU.bypass,
                                        replica_groups=rg,
                                        ins=[catL[l + 1][0 : CROWS[0], :]],
                                        outs=[catG[l + 1][0][:]],
                                    )
                        else:
                            nc.vector.tensor_tensor(
                                out=htile[:].rearrange("p (h d) -> p h d", h=H),
                                in0=ps[:, 0:FD].rearrange("p (h d) -> p h d", h=H),
                                in1=rv, op=ALU.mult,
                            )
                            nc.sync.dma_start(
                                out=out3[b * P : (b + 1) * P, :], in_=htile[:]
                            )
                    if prep:
                        nc.gpsimd.collective_compute(
                            "AllGather", ALU.bypass, replica_groups=rg,
                            ins=[catL[l + 1][CROWS[0] : R, :]],
                            outs=[catG[l + 1][1][:]],
                        )

            edge_pass(0)
            if nlayers > 1:
                edge_pass(1)
            if nlayers > 2:
                edge_pass(2)
            if dbg:
                nc.sync.dma_start(out=dbg_t["d_nh"][:], in_=nh_sliceT[:])

    nc.compile()
    return nc


_CACHE = {}


def get_program(cfg):
    key = (cfg.TA, cfg.TB, cfg.odf, cfg.tc)
    if key not in _CACHE:
        _CACHE[key] = build_program(cfg)
    return _CACHE[key]


def run(inputs, trace=False):
    from concourse.bass_utils import run_bass_kernel_spmd

    percore, cfg = host_prep(inputs)
    nc = get_program(cfg)
    res = run_bass_kernel_spmd(nc, percore, list(range(NDEV)), trace=trace)
    outs = [res.results[i]["out3"] for i in range(NDEV)]
    full = np.concatenate(outs, axis=0)  # [NPAD, 96]
    return full, res


def kernel(**inputs) -> np.ndarray:
    full, _ = run(inputs)
    idxs = np.asarray(inputs["idxs"]).astype(np.int64)
    return np.ascontiguousarray(full[idxs]).astype(np.float32)


# revision 55
# speedup vs baseline: 4.7722x; 1.0074x over previous
"""Trainium2 Bass kernel for 3-layer EGAT message passing (nn_COUNTYOD).

Strategy (8 cores, edge parallelism by dst range):
  - Host: sort edges by dst; device d owns dst nodes [d*6272, (d+1)*6272);
    edges grouped into 49 blocks of 128 dst nodes. Within a block, edges are
    split into two groups by src chunk (first 24 / last 25 blocks of the
    src's owner device), each padded to a tile multiple (TA/TB tiles).
  - Gather tables catA/catB hold rows [nh@Wni + b | nh@Wns] (192 f32 =
    768B, %256==0) in chunk-major order, so a whole block's src rows are
    fetched with TWO dma_gather instructions (int16 idxs, -1 suffix pad,
    per-device counts via reg_load) instead of per-tile indirect DMAs.
  - nj[dst] is block-local: njb = nh_sliceT[block] @ Wnj once per block,
    then selected per tile with a transposed one-hot matmul on PE.
  - Single fused pass per block: f_pre (PSUM) -> leaky (Act Prelu) ->
    logits (mul+grouped reduce) -> ex=exp(min(e,60)) -> msg=ns*ex (bf16)
    -> one-hot scatter matmul accumulating [h | s] in PSUM -> h = ps/s.
  - Edge features stored row-major bf16 [E,96]; next layer loads them
    transposed via dma_start_transpose. All per-tile matmuls in bf16.
  - Layer-0 tables are host-precomputed params (no collective); layers 1-2
    AllGather the locally computed 192-col table in two chunks.
"""

import sys
import numpy as np

for _p in ("/opt/trn_rl_repo",):
    if _p not in sys.path:
        sys.path.insert(0, _p)

import ml_dtypes  # noqa: E402
import concourse.bass as bass  # noqa: E402
import concourse.bacc as bacc  # noqa: E402
import concourse.mybir as mybir  # noqa: E402
import concourse.tile as tile  # noqa: E402
from concourse.masks import make_identity  # noqa: E402

F32 = mybir.dt.float32
BF16 = mybir.dt.bfloat16
I32 = mybir.dt.int32
I16 = mybir.dt.int16
AF = mybir.ActivationFunctionType
ALU = mybir.AluOpType
BNP = ml_dtypes.bfloat16

P = 128
FD = 96           # H*HE
CW = 192          # gather row width [ni|ns]
H = 3
HE = 32
EPS = 1e-20
N_REAL = 50000
NDEV = 8
NBLK = 49
R = NBLK * P      # 6272 nodes per device
NPAD = NDEV * R
CBLK = (24, 25)   # blocks per src chunk (within device)
CROWS = (CBLK[0] * P, CBLK[1] * P)          # 3072, 3200
CTOT = (NDEV * CROWS[0], NDEV * CROWS[1])   # 24576, 25600 rows per table


GSUB = 8          # max tiles (1024 idxs) per dma_gather (SWDGE ring limit)


def _subs(Tn):
    """Split Tn tiles into sub-gather spans of <= GSUB tiles."""
    out = []
    t0 = 0
    while t0 < Tn:
        out.append((t0, min(GSUB, Tn - t0)))
        t0 += GSUB
    return out


class Cfg:
    def __init__(self, TA, TB, odf, tc=4):
        self.TA = TA
        self.TB = TB
        self.T = TA + TB
        self.odf = odf
        self.tc = tc
        self.ttot = NBLK * self.T
        self.epd = self.ttot * P
        self.subsA = _subs(TA)
        self.subsB = _subs(TB)
        self.nsub = len(self.subsA) + len(self.subsB)


def _wrap16(v):
    """idx list -> [16, n/16] layout (value i at [i%16, i//16])."""
    return np.ascontiguousarray(v.reshape(-1, 16).T)


def host_prep(inputs, cfg=None):
    src = np.asarray(inputs["src"]).astype(np.int64)
    dst = np.asarray(inputs["dst"]).astype(np.int64)
    E = src.shape[0]

    order = np.argsort(dst, kind="stable")
    ssrc, sdst = src[order], dst[order]
    ef0 = np.asarray(inputs["countyodfeats"]).astype(np.float32)[order]
    odf = ef0.shape[1]

    # chunk id + table row for each (sorted) edge's src
    d_s = ssrc // R
    r_s = ssrc % R
    k_s = (r_s >= CROWS[0]).astype(np.int64)
    row_s = np.where(k_s == 0, d_s * CROWS[0] + r_s,
                     d_s * CROWS[1] + (r_s - CROWS[0]))

    blk = sdst // P
    blkcnt = np.bincount(blk, minlength=NPAD // P)
    starts = np.zeros(NPAD // P + 1, np.int64)
    np.cumsum(blkcnt, out=starts[1:])

    # per (dev, blk) group sizes to fix TA/TB
    nlo = np.zeros((NDEV, NBLK), np.int64)
    nhi = np.zeros((NDEV, NBLK), np.int64)
    for d in range(NDEV):
        for b in range(NBLK):
            g = d * NBLK + b
            ks = k_s[starts[g]:starts[g + 1]]
            nlo[d, b] = int((ks == 0).sum())
            nhi[d, b] = int((ks == 1).sum())
    TA = max(1, int(np.ceil(nlo.max() / P)))
    TB = max(1, int(np.ceil(nhi.max() / P)))
    if cfg is None:
        cfg = Cfg(TA, TB, odf)
    else:
        assert cfg.TA >= TA and cfg.TB >= TB
    T, TP = cfg.T, cfg.T * P

    def subcnt(n, t0, tn):
        """valid count within sub-gather span [t0*P, t0*P+tn*P) of a
        region with n valid prefix entries; >=1 (a forced idx 0 covers 0)."""
        return max(1, min(n - t0 * P, tn * P)) if n > t0 * P else 1

    percore = []
    for d in range(NDEV):
        idxA = np.full((NBLK, cfg.TA * P), -1, np.int64)
        idxB = np.full((NBLK, cfg.TB * P), -1, np.int64)
        cnts = np.zeros((1, cfg.nsub * NBLK), np.int32)
        dloc = np.full((NBLK, TP), -1.0, np.float32)
        efp = np.zeros((NBLK, TP, odf), np.float32)
        for b in range(NBLK):
            g = d * NBLK + b
            s0, s1 = starts[g], starts[g + 1]
            ks = k_s[s0:s1]
            lo = np.nonzero(ks == 0)[0]
            hi = np.nonzero(ks == 1)[0]
            na, nb = len(lo), len(hi)
            idxA[b, :na] = row_s[s0:s1][lo]
            idxB[b, :nb] = row_s[s0:s1][hi]
            ci = b * cfg.nsub
            for (t0, tn) in cfg.subsA:
                cnts[0, ci] = subcnt(na, t0, tn)
                if na <= t0 * P:
                    idxA[b, t0 * P] = 0
                ci += 1
            for (t0, tn) in cfg.subsB:
                cnts[0, ci] = subcnt(nb, t0, tn)
                if nb <= t0 * P:
                    idxB[b, t0 * P] = 0
                ci += 1
            dloc[b, :na] = (sdst[s0:s1][lo] - g * P).astype(np.float32)
            dloc[b, cfg.TA * P : cfg.TA * P + nb] = (
                sdst[s0:s1][hi] - g * P
            ).astype(np.float32)
            efp[b, :na] = ef0[s0:s1][lo]
            efp[b, cfg.TA * P : cfg.TA * P + nb] = ef0[s0:s1][hi]
        ia = np.concatenate([_wrap16(idxA[b]) for b in range(NBLK)], axis=1)
        ib = np.concatenate([_wrap16(idxB[b]) for b in range(NBLK)], axis=1)
        dstloc = dloc.reshape(-1, P).T            # [128, ttot]
        dstlocT = dloc.reshape(1, -1)             # [1, epd]
        percore.append(
            dict(
                idxA=np.ascontiguousarray(np.tile(ia, (8, 1)).astype(np.int16)),
                idxB=np.ascontiguousarray(np.tile(ib, (8, 1)).astype(np.int16)),
                cnts=cnts,
                dstloc=np.ascontiguousarray(dstloc.astype(BNP)),
                dstlocT=np.ascontiguousarray(dstlocT.astype(BNP)),
                ef0T=np.ascontiguousarray(
                    efp.reshape(-1, odf).T.astype(BNP)
                ),
            )
        )

    g = lambda k: np.asarray(inputs[k]).astype(np.float32)
    nh0 = g("ndata_h")
    nh0p = np.zeros((NPAD, FD), np.float32)
    nh0p[:N_REAL] = nh0[:N_REAL]
    for d in range(NDEV):
        sl = nh0p[d * R : (d + 1) * R]
        percore[d]["nh0T"] = np.ascontiguousarray(
            np.concatenate([sl.T, np.ones((1, R), np.float32)], axis=0)
        )

    # host-precomputed layer-0 gather tables (chunk-major row order)
    ni0 = nh0p @ g("Wni0") + g("bias0")[None, :]
    ns0 = nh0p @ g("Wns0")
    cat0 = np.concatenate([ni0, ns0], axis=1).astype(np.float32)  # [NPAD,192]
    c4 = cat0.reshape(NDEV, R, CW)
    catA0 = np.ascontiguousarray(c4[:, : CROWS[0]].reshape(CTOT[0], CW))
    catB0 = np.ascontiguousarray(c4[:, CROWS[0] :].reshape(CTOT[1], CW))

    def wcat(Wni, Wns, bias):
        w = np.concatenate([Wni, Wns], axis=1)
        b = np.concatenate([bias, np.zeros(FD, np.float32)])[None, :]
        return np.ascontiguousarray(
            np.concatenate([w, b], axis=0).astype(np.float32)
        )

    def wnj(W):
        return np.ascontiguousarray(
            np.concatenate([W, np.zeros((1, FD), np.float32)], axis=0)
        )

    weights = dict(
        catA0=catA0,
        catB0=catB0,
        wcat1=wcat(g("Wni")[0], g("Wns")[0], g("bias")[0]),
        wcat2=wcat(g("Wni")[1], g("Wns")[1], g("bias")[1]),
        wnj0=wnj(g("Wnj0")),
        wnj1=wnj(g("Wnj")[0]),
        wnj2=wnj(g("Wnj")[1]),
        wfij0=np.ascontiguousarray(g("Wfij0").astype(BNP)),
        # layers 1-2 contract over the padded 128-row ef layout
        wfij1=np.ascontiguousarray(
            np.concatenate(
                [g("Wfij")[0], np.zeros((P - FD, FD), np.float32)], axis=0
            ).astype(BNP)
        ),
        wfij2=np.ascontiguousarray(
            np.concatenate(
                [g("Wfij")[1], np.zeros((P - FD, FD), np.float32)], axis=0
            ).astype(BNP)
        ),
        attn0=np.ascontiguousarray(np.repeat(g("attn0").reshape(1, FD), P, 0)),
        attn1=np.ascontiguousarray(
            np.repeat(g("attn").reshape(2, FD)[0:1], P, 0)
        ),
        attn2=np.ascontiguousarray(
            np.repeat(g("attn").reshape(2, FD)[1:2], P, 0)
        ),
    )
    for d in range(NDEV):
        percore[d].update(weights)
    return percore, cfg


def build_program(cfg, nlayers=3, dbg=False, stage=99):
    nc = bacc.Bacc("TRN2", target_bir_lowering=False, debug=False)
    c = cfg
    T, TA, TB, TTOT, EPD = c.T, c.TA, c.TB, c.ttot, c.epd

    # ---- I/O ----
    pr = {}
    pr["nh0T"] = nc.declare_dram_parameter("nh0T", [FD + 1, R], F32, isOutput=False)
    pr["ef0T"] = nc.declare_dram_parameter("ef0T", [c.odf, EPD], BF16, isOutput=False)
    pr["idxA"] = nc.declare_dram_parameter("idxA", [P, NBLK * TA * 8], I16, isOutput=False)
    pr["idxB"] = nc.declare_dram_parameter("idxB", [P, NBLK * TB * 8], I16, isOutput=False)
    pr["cnts"] = nc.declare_dram_parameter("cnts", [1, cfg.nsub * NBLK], I32, isOutput=False)
    pr["dstloc"] = nc.declare_dram_parameter("dstloc", [P, TTOT], BF16, isOutput=False)
    pr["dstlocT"] = nc.declare_dram_parameter("dstlocT", [1, EPD], BF16, isOutput=False)
    pr["catA0"] = nc.declare_dram_parameter("catA0", [CTOT[0], CW], F32, isOutput=False)
    pr["catB0"] = nc.declare_dram_parameter("catB0", [CTOT[1], CW], F32, isOutput=False)
    for i in (1, 2):
        pr[f"wcat{i}"] = nc.declare_dram_parameter(f"wcat{i}", [FD + 1, CW], F32, isOutput=False)
    for i in range(3):
        pr[f"wnj{i}"] = nc.declare_dram_parameter(f"wnj{i}", [FD + 1, FD], F32, isOutput=False)
        pr[f"attn{i}"] = nc.declare_dram_parameter(f"attn{i}", [P, FD], F32, isOutput=False)
    pr["wfij0"] = nc.declare_dram_parameter("wfij0", [c.odf, FD], BF16, isOutput=False)
    pr["wfij1"] = nc.declare_dram_parameter("wfij1", [P, FD], BF16, isOutput=False)
    pr["wfij2"] = nc.declare_dram_parameter("wfij2", [P, FD], BF16, isOutput=False)
    out3 = nc.declare_dram_parameter("out3", [R, FD], F32, isOutput=True)
    dbg_t = {}
    if dbg:
        dbg_t["d_nh"] = nc.declare_dram_parameter("d_nh", [FD + 1, R], F32, isOutput=True)
        dbg_t["d_G"] = nc.declare_dram_parameter("d_G", [P, cfg.T * CW], F32, isOutput=True)
        dbg_t["d_eL"] = nc.declare_dram_parameter("d_eL", [P, cfg.T * H], F32, isOutput=True)
        dbg_t["d_t2"] = nc.declare_dram_parameter("d_t2", [P, cfg.T * FD], F32, isOutput=True)

    # ---- internal DRAM ----
    efA = nc.dram_tensor("efA", [EPD, P], BF16)
    efB = nc.dram_tensor("efB", [EPD, P], BF16)
    catL = {
        1: nc.dram_tensor("catL1", [R, CW], F32),
        2: nc.dram_tensor("catL2", [R, CW], F32),
    }
    catG = {
        1: (
            nc.dram_tensor("catGA1", [CTOT[0], CW], F32, addr_space="Shared"),
            nc.dram_tensor("catGB1", [CTOT[1], CW], F32, addr_space="Shared"),
        ),
        2: (
            nc.dram_tensor("catGA2", [CTOT[0], CW], F32, addr_space="Shared"),
            nc.dram_tensor("catGB2", [CTOT[1], CW], F32, addr_space="Shared"),
        ),
    }

    rg = [list(range(NDEV))]

    with tile.TileContext(nc) as tc:
        with tc.tile_pool(name="persist", bufs=1) as pp:
            ident = pp.tile([P, P], F32, tag="ident")
            make_identity(nc, ident[:])
            iota_i = pp.tile([P, P], I32, tag="iota_i")
            nc.gpsimd.iota(iota_i[:], pattern=[[1, P]], base=0, channel_multiplier=0)
            iota_f = pp.tile([P, P], BF16, tag="iota_f")
            nc.vector.tensor_copy(out=iota_f[:], in_=iota_i[:])
            iota_ci = pp.tile([P, 1], I32, tag="iota_ci")
            nc.gpsimd.iota(iota_ci[:], pattern=[[0, 1]], base=0, channel_multiplier=1)
            iota_c = pp.tile([P, 1], F32, tag="iota_c")
            nc.vector.tensor_copy(out=iota_c[:], in_=iota_ci[:])
            ones1 = pp.tile([1, P], BF16, tag="ones1")
            nc.vector.memset(ones1[:], 1.0)

            idxA = pp.tile([P, NBLK * TA * 8], I16, tag="idxA")
            idxB = pp.tile([P, NBLK * TB * 8], I16, tag="idxB")
            cnts = pp.tile([1, cfg.nsub * NBLK], I32, tag="cnts")
            dstloc = pp.tile([P, TTOT], BF16, tag="dstloc")
            nc.sync.dma_start(out=idxA[:], in_=pr["idxA"][:])
            nc.sync.dma_start(out=idxB[:], in_=pr["idxB"][:])
            nc.sync.dma_start(out=cnts[:], in_=pr["cnts"][:])
            nc.sync.dma_start(out=dstloc[:], in_=pr["dstloc"][:])

            wcat_sb = {}
            for i in (1, 2):
                w = pp.tile([FD + 1, CW], F32, tag=f"wcat{i}")
                nc.sync.dma_start(out=w[:], in_=pr[f"wcat{i}"][:])
                wcat_sb[i] = w
            wnj_sb, attn_sb, wfij_sb = [], [], []
            for i in range(3):
                w = pp.tile([FD + 1, FD], F32, tag=f"wnj{i}")
                nc.sync.dma_start(out=w[:], in_=pr[f"wnj{i}"][:])
                wnj_sb.append(w)
                a = pp.tile([P, FD], F32, tag=f"attn{i}")
                nc.sync.dma_start(out=a[:], in_=pr[f"attn{i}"][:])
                attn_sb.append(a)
                cdim = c.odf if i == 0 else P
                w2 = pp.tile([cdim, FD], BF16, tag=f"wfij{i}")
                nc.sync.dma_start(out=w2[:], in_=pr[f"wfij{i}"][:])
                wfij_sb.append(w2)

            nh_sliceT = pp.tile([FD + 1, R], F32, tag="nh_sliceT")
            nc.sync.dma_start(out=nh_sliceT[:], in_=pr["nh0T"][:])

            # gather destination tiles: manually double-buffered and zeroed
            # once, so pad slots (skipped by the gather) always hold finite
            # values for the zero one-hot columns to nullify.
            G0 = pp.tile([P, T * CW], F32, tag="G0")
            G1 = pp.tile([P, T * CW], F32, tag="G1")
            G_bufs = [G0, G1]
            nc.vector.memset(G_bufs[0][:], 0.0)
            nc.vector.memset(G_bufs[1][:], 0.0)
            # stored edge features, padded to 128 cols; pads stay zero
            fs0 = pp.tile([P, T * P], BF16, tag="fs0")
            fs1 = pp.tile([P, T * P], BF16, tag="fs1")
            fs_bufs = [fs0, fs1]
            nc.vector.memset(fs_bufs[0][:], 0.0)
            nc.vector.memset(fs_bufs[1][:], 0.0)

            ra = nc.gpsimd.alloc_register("ra")
            rb = nc.gpsimd.alloc_register("rb")

            def ap(t, offset, pattern):
                v = t[:]
                return bass.AP(v.tensor, v.offset + offset, pattern)

            def edge_pass(l):
                cdim = c.odf if l == 0 else P
                wf = wfij_sb[l]
                attn = attn_sb[l]
                wnj = wnj_sb[l]
                catA_t = pr["catA0"] if l == 0 else catG[l][0]
                catB_t = pr["catB0"] if l == 0 else catG[l][1]
                ef_dst = efA if l == 0 else (efB if l == 1 else None)
                ef_rows = efA if l == 1 else efB  # row-major source (l>=1)
                prep = (l < 2) and (nlayers > l + 1) and stage > 5

                with (
                    tc.tile_pool(name="pb", bufs=2) as pb,
                    tc.tile_pool(name="pc", bufs=3) as pc,
                    tc.tile_pool(name="qrep", bufs=1, space="PSUM") as qrep,
                    tc.tile_pool(name="qfp", bufs=2, space="PSUM") as qfp,
                    tc.tile_pool(name="qps", bufs=2, space="PSUM") as qps,
                    tc.tile_pool(name="qnh", bufs=1, space="PSUM") as qnh,
                ):
                    for b in range(NBLK):
                        # ---- gathers: G = [ni|ns] rows of this block ----
                        G = G_bufs[b % 2]
                        ci = b * cfg.nsub
                        for gi, (idxt, Tbase, Toff, subs, cat_t) in enumerate(
                            [
                                (idxA, TA, 0, cfg.subsA, catA_t),
                                (idxB, TB, TA, cfg.subsB, catB_t),
                            ]
                        ):
                            for (t0, tn) in subs:
                                nc.gpsimd.reg_load(ra, cnts[0:1, ci : ci + 1])
                                ci += 1
                                nc.gpsimd.dma_gather(
                                    out_ap=ap(
                                        G, (Toff + t0) * CW,
                                        [G[:].ap[0], [CW, tn], [1, CW]],
                                    ),
                                    in_ap=cat_t[:, :],
                                    idxs_ap=idxt[
                                        :,
                                        b * Tbase * 8 + t0 * 8 :
                                        b * Tbase * 8 + (t0 + tn) * 8,
                                    ],
                                    num_idxs=tn * P, num_idxs_reg=ra,
                                    elem_size=CW,
                                )

                        if stage <= 1:
                            htile = pb.tile([P, FD], F32, tag="htile")
                            nc.vector.tensor_copy(out=htile[:], in_=G[:, 0:FD])
                            nc.sync.dma_start(
                                out=out3[b * P : (b + 1) * P, :], in_=htile[:]
                            )
                            continue

                        # ---- block-local tables ----
                        dT = pb.tile([1, T * P], BF16, tag="dT")
                        nc.sync.dma_start(
                            out=dT[:], in_=pr["dstlocT"][:, b * T * P : (b + 1) * T * P]
                        )
                        njp = qnh.tile([P, FD], F32, tag="njp")
                        nc.tensor.matmul(
                            out=njp[:], lhsT=nh_sliceT[:, b * P : (b + 1) * P],
                            rhs=wnj[:], start=True, stop=True,
                        )
                        njb = pb.tile([P, FD], BF16, tag="njb")
                        nc.scalar.activation(out=njb[:], in_=njp[:], func=AF.Copy)

                        efc = pc.tile([cdim, T * P], BF16, tag="efc")
                        if l == 0:
                            nc.sync.dma_start(
                                out=efc[:],
                                in_=pr["ef0T"][:, b * T * P : (b + 1) * T * P],
                            )
                        else:
                            nc.sync.dma_start_transpose(
                                efc[:],
                                ef_rows[b * T * P : (b + 1) * T * P, :],
                            )

                        oh = pb.tile([P, T * P], BF16, tag="oh")
                        ohT = pb.tile([P, T * P], BF16, tag="ohT")
                        t2 = pb.tile([P, T * FD], F32, tag="t2")

                        for c0 in range(0, T, c.tc):
                            tcn = min(c.tc, T - c0)
                            # one-hots for the chunk (edge-major on DVE;
                            # node-major via PE broadcast of dT then DVE)
                            nc.vector.tensor_tensor(
                                out=ap(oh, c0 * P, [oh[:].ap[0], [P, tcn], [1, P]]),
                                in0=ap(dstloc, b * T + c0, [dstloc[:].ap[0], [1, tcn], [0, P]]),
                                in1=ap(iota_f, 0, [iota_f[:].ap[0], [0, tcn], [1, P]]),
                                op=ALU.is_equal,
                            )
                            dRep = qrep.tile([P, c.tc * P], F32, tag="dRep")
                            nc.tensor.matmul(
                                out=dRep[:, 0 : tcn * P],
                                lhsT=ones1[:],
                                rhs=dT[:, c0 * P : (c0 + tcn) * P],
                                start=True, stop=True,
                            )
                            nc.vector.tensor_tensor(
                                out=ap(ohT, c0 * P, [ohT[:].ap[0], [P, tcn], [1, P]]),
                                in0=ap(iota_c, 0, [iota_c[:].ap[0], [0, tcn], [0, P]]),
                                in1=ap(dRep, 0, [dRep[:].ap[0], [P, tcn], [1, P]]),
                                op=ALU.is_equal,
                            )
                            fp = qfp.tile([P, c.tc * FD], F32, tag="fp")
                            for t in range(tcn):
                                nc.tensor.matmul(
                                    out=fp[:, t * FD : (t + 1) * FD],
                                    lhsT=ohT[:, (c0 + t) * P : (c0 + t + 1) * P],
                                    rhs=njb[:],
                                    start=True, stop=False,
                                    skip_group_check=True,
                                )
                                nc.tensor.matmul(
                                    out=fp[:, t * FD : (t + 1) * FD],
                                    lhsT=efc[:, (c0 + t) * P : (c0 + t + 1) * P],
                                    rhs=wf[:],
                                    start=False, stop=True,
                                    skip_group_check=True,
                                )
                            # t2 = fp + ni
                            nc.vector.tensor_tensor(
                                out=ap(t2, c0 * FD, [t2[:].ap[0], [FD, tcn], [1, FD]]),
                                in0=fp[:, 0 : tcn * FD].rearrange(
                                    "p (t f) -> p t f", t=tcn
                                ),
                                in1=ap(G, c0 * CW, [G[:].ap[0], [CW, tcn], [1, FD]]),
                                op=ALU.add,
                            )

                        if stage <= 2:
                            htile = pb.tile([P, FD], F32, tag="htile")
                            nc.vector.tensor_copy(out=htile[:], in_=t2[:, 0:FD])
                            nc.sync.dma_start(
                                out=out3[b * P : (b + 1) * P, :], in_=htile[:]
                            )
                            continue

                        # ---- leaky + logits + softmax numerator ----
                        fl = pb.tile([P, T * FD], F32, tag="fl")
                        nc.vector.scalar_tensor_tensor(
                            out=fl[:], in0=t2[:], scalar=0.01, in1=t2[:],
                            op0=ALU.mult, op1=ALU.max,
                        )
                        # logit products overwrite G's ni columns (dead after t2)
                        nc.vector.tensor_tensor(
                            out=ap(G, 0, [G[:].ap[0], [CW, T], [1, FD]]),
                            in0=fl[:].rearrange("p (t f) -> p t f", t=T),
                            in1=ap(attn, 0, [attn[:].ap[0], [0, T], [1, FD]]),
                            op=ALU.mult,
                        )
                        eL = pb.tile([P, T * H], F32, tag="eL")
                        nc.vector.tensor_reduce(
                            out=eL[:].rearrange("p (t h) -> p t h", t=T),
                            in_=ap(G, 0, [G[:].ap[0], [CW, T], [HE, H], [1, HE]]),
                            axis=mybir.AxisListType.X, op=ALU.add,
                        )
                        nc.vector.tensor_scalar(
                            out=eL[:], in0=eL[:], scalar1=60.0, scalar2=None,
                            op0=ALU.min,
                        )
                        nc.scalar.activation(out=eL[:], in_=eL[:], func=AF.Exp)
                        if dbg and b == 0 and l == 0:
                            nc.sync.dma_start(out=dbg_t["d_G"][:], in_=G[:])
                            nc.sync.dma_start(out=dbg_t["d_eL"][:], in_=eL[:])
                            nc.sync.dma_start(out=dbg_t["d_t2"][:], in_=t2[:])

                        if stage <= 3:
                            htile = pb.tile([P, FD], F32, tag="htile")
                            nc.vector.tensor_copy(out=htile[:], in_=fl[:, 0:FD])
                            nc.sync.dma_start(
                                out=out3[b * P : (b + 1) * P, :], in_=htile[:]
                            )
                            continue

                        # ---- messages (bf16) ----
                        me = pb.tile([P, T * (FD + H)], BF16, tag="me")
                        nc.gpsimd.tensor_tensor(
                            out=ap(me, 0, [me[:].ap[0], [FD + H, T], [HE, H], [1, HE]]),
                            in0=ap(G, FD, [G[:].ap[0], [CW, T], [HE, H], [1, HE]]),
                            in1=ap(eL, 0, [eL[:].ap[0], [H, T], [1, H], [0, HE]]),
                            op=ALU.mult,
                        )
                        nc.scalar.activation(
                            out=ap(me, FD, [me[:].ap[0], [FD + H, T], [1, H]]),
                            in_=eL[:].rearrange("p (t h) -> p t h", t=T),
                            func=AF.Copy,
                        )

                        # ---- scatter [h|s] ----
                        ps = qps.tile([P, FD + H], F32, tag="ps")
                        for t in range(T):
                            nc.tensor.matmul(
                                out=ps[:],
                                lhsT=oh[:, t * P : (t + 1) * P],
                                rhs=me[:, t * (FD + H) : (t + 1) * (FD + H)],
                                start=(t == 0), stop=(t == T - 1),
                                skip_group_check=True,
                            )

                        if stage <= 4:
                            htile = pb.tile([P, FD], F32, tag="htile")
                            nc.scalar.activation(
                                out=htile[:], in_=ps[:, 0:FD], func=AF.Copy
                            )
                            nc.sync.dma_start(
                                out=out3[b * P : (b + 1) * P, :], in_=htile[:]
                            )
                            continue

                        # ---- store edge features for next layer ----
                        if stage > 5 and ef_dst is not None:
                            fs = fs_bufs[b % 2]
                            nc.scalar.activation(
                                out=ap(fs, 0, [fs[:].ap[0], [P, T], [1, FD]]),
                                in_=t2[:].rearrange("p (t f) -> p t f", t=T),
                                func=AF.Relu,
                            )
                            efout = bass.AP(
                                ef_dst[:].tensor,
                                ef_dst[:].offset + b * T * P * P,
                                [[P, P], [P * P, T], [1, P]],
                            )
                            nc.sync.dma_start(
                                out=efout,
                                in_=fs[:].rearrange("p (t f) -> p t f", t=T),
                            )

                        # ---- h = ps/s ----
                        sp = pb.tile([P, H], F32, tag="sp")
                        nc.vector.tensor_scalar_add(
                            out=sp[:], in0=ps[:, FD : FD + H], scalar1=EPS
                        )
                        rcp = pb.tile([P, H], F32, tag="rcp")
                        nc.vector.reciprocal_approx_fast(out=rcp[:], in_=sp[:])
                        rv = ap(rcp, 0, [rcp[:].ap[0], [1, H], [0, HE]])
                        htile = pb.tile([P, FD], F32, tag="htile")
                        if l < 2:
                            hr = pb.tile([P, FD], F32, tag="hr")
                            nc.scalar.activation(
                                out=hr[:], in_=ps[:, 0:FD], func=AF.Relu
                            )
                            nc.vector.tensor_tensor(
                                out=htile[:].rearrange("p (h d) -> p h d", h=H),
                                in0=hr[:].rearrange("p (h d) -> p h d", h=H),
                                in1=rv, op=ALU.mult,
                            )
                            hT = qnh.tile([FD, P], F32, tag="hT")
                            nc.tensor.transpose(
                                out=hT[:], in_=htile[:], identity=ident[:]
                            )
                            nc.scalar.activation(
                                out=nh_sliceT[0:FD, b * P : (b + 1) * P],
                                in_=hT[:], func=AF.Copy,
                            )
                            if prep:
                                # next layer's gather-table rows for this block
                                pt = qnh.tile([P, CW], F32, tag="pt")
                                nc.tensor.matmul(
                                    out=pt[:],
                                    lhsT=nh_sliceT[:, b * P : (b + 1) * P],
                                    rhs=wcat_sb[l + 1][:],
                                    start=True, stop=True,
                                )
                                cs = pb.tile([P, CW], F32, tag="cs")
                                nc.scalar.activation(
                                    out=cs[:], in_=pt[:], func=AF.Copy
                                )
                                nc.sync.dma_start(
                                    out=catL[l + 1][b * P : (b + 1) * P, :],
                                    in_=cs[:],
                                )
                                if b == CBLK[0] - 1:
                                    nc.gpsimd.collective_compute(
                                        "AllGather", ALU.bypass,
                                        replica_groups=rg,
                                        ins=[catL[l + 1][0 : CROWS[0], :]],
                                        outs=[catG[l + 1][0][:]],
                                    )
                        else:
                            nc.vector.tensor_tensor(
                                out=htile[:].rearrange("p (h d) -> p h d", h=H),
                                in0=ps[:, 0:FD].rearrange("p (h d) -> p h d", h=H),
                                in1=rv, op=ALU.mult,
                            )
                            nc.sync.dma_start(
                                out=out3[b * P : (b + 1) * P, :], in_=htile[:]
                            )
                    if prep:
                        nc.gpsimd.collective_compute(
                            "AllGather", ALU.bypass, replica_groups=rg,
                            ins=[catL[l + 1][CROWS[0] : R, :]],
                            outs=[catG[l + 1][1][:]],
                        )

            edge_pass(0)
            if nlayers > 1:
                edge_pass(1)
            if nlayers > 2:
                edge_pass(2)
            if dbg:
                nc.sync.dma_start(out=dbg_t["d_nh"][:], in_=nh_sliceT[:])

    nc.compile()
    return nc


_CACHE = {}


def get_program(cfg):
    key = (cfg.TA, cfg.TB, cfg.odf, cfg.tc)
    if key not in _CACHE:
        _CACHE[key] = build_program(cfg)
    return _CACHE[key]


def run(inputs, trace=False):
    from concourse.bass_utils import run_bass_kernel_spmd

    percore, cfg = host_prep(inputs)
    nc = get_program(cfg)
    res = run_bass_kernel_spmd(nc, percore, list(range(NDEV)), trace=trace)
    outs = [res.results[i]["out3"] for i in range(NDEV)]
    full = np.concatenate(outs, axis=0)  # [NPAD, 96]
    return full, res


def kernel(**inputs) -> np.ndarray:
    full, _ = run(inputs)
    idxs = np.asarray(inputs["idxs"]).astype(np.int64)
    return np.ascontiguousarray(full[idxs]).astype(np.float32)
